# revision 6
# baseline (speedup 1.0000x reference)
"""Attention-pooling kernel for 8 Trainium2 NeuronCores (SPMD, data-parallel over N).

Reference computation (fp32):
    h      = tanh(embeddings @ W_V.T)        [N, 512]
    scores = h @ w                           [N]
    p      = softmax(scores)                 [N]
    attn   = p @ embeddings                  [1024]
    returns (attn, p)

Strategy:
  - Shard N=32768 across 8 cores (4096 rows each).
  - Host pre-transposes each shard to E^T (d on partitions) so the score
    matmul needs no on-chip transpose; host pre-rounds PE operands to TF32
    (float32r) so the PE runs at full rate (1 cycle/row vs 4 for fp32).
  - Per core, per 512-instance block:
      h^T[v,i] = sum_d W_V^T[d,v] E^T[d,i]   PE, f32r, PSUM-accumulated
      tanh                                    ACT (psum -> sbuf, f32r out)
      scores[1,i] = sum_v w[v] h^T[v,i]       PE, f32r
      p = exp(scores)                         ACT
      p broadcast across 128 partitions       PE outer product with ones
      prod[d,i] = E^T[d,i] * p[i]             DVE
      partial[d] += sum_i prod[d,i]           ACT copy with accum_out
  - No collectives: each core returns its unnormalized exp(scores) slice and
    partial pooled sums; the tiny final reduction (8 x 1025 floats) happens
    on host during the gather step.
"""

import numpy as np

import concourse.bacc as bacc
import concourse.mybir as mybir
import concourse.tile as tile
from concourse.bass_utils import run_bass_kernel_spmd

F32 = mybir.dt.float32
F32R = mybir.dt.float32r

N = 32768
D = 1024
V = 512
NCORES = 8
NC_ROWS = N // NCORES          # 4096 instances per core
NBLK = 8                       # i-blocks per core
BLK = NC_ROWS // NBLK          # 512 instances per block
VT = V // 128                  # 4 v-tiles
DC = D // 128                  # 8 d-chunks


def _tf32_round(x: np.ndarray) -> np.ndarray:
    """Round fp32 to TF32 (float32r) bit patterns, round-to-nearest-even."""
    u = np.ascontiguousarray(x, dtype=np.float32).view(np.uint32).copy()
    lsb = (u >> np.uint32(13)) & np.uint32(1)
    u += np.uint32(0x0FFF) + lsb
    u &= np.uint32(0xFFFFE000)
    return u.view(np.float32)


def build_program(replicate: int = 1, loop_iters: int = 1):
    """Build the SPMD single-core program (same program on all 8 cores).

    replicate > 1 unrolls the whole compute body; loop_iters > 1 wraps the
    body in a hardware For_i loop (same inputs, same outputs each iteration).
    Both are identity transforms on the output, used only for timing.
    """
    nc = bacc.Bacc("TRN2", target_bir_lowering=False, debug=False,
                   num_devices=NCORES)

    etr_d = nc.dram_tensor("etr", [NBLK, 128, DC * BLK], F32R,
                           kind="ExternalInput").ap()
    wvt_d = nc.dram_tensor("wvt", [128, DC * V], F32R,
                           kind="ExternalInput").ap()
    wch_d = nc.dram_tensor("wch", [128, VT], F32R, kind="ExternalInput").ap()
    ones_d = nc.dram_tensor("ones", [1, 128], F32R, kind="ExternalInput").ap()
    pool_d = nc.dram_tensor("pool", [128, DC], F32,
                            kind="ExternalOutput").ap()
    pw_d = nc.dram_tensor("pw", [NBLK, BLK], F32, kind="ExternalOutput").ap()

    with tile.TileContext(nc) as tc:
        with (
            tc.tile_pool(name="const", bufs=1) as constp,
            tc.tile_pool(name="et", bufs=4) as etp,
            tc.tile_pool(name="tanh", bufs=6) as tanhp,
            tc.tile_pool(name="scr", bufs=2) as scrp,
            tc.tile_pool(name="sexp", bufs=2) as sexpp,
            tc.tile_pool(name="fin", bufs=1) as finp,
            tc.tile_pool(name="hps", bufs=3, space="PSUM") as hp,
            tc.tile_pool(name="scps", bufs=2, space="PSUM") as scp,
            tc.tile_pool(name="pbcps", bufs=2, space="PSUM") as pbcp,
        ):
            wvt = constp.tile([128, DC, V], F32R, tag="wvt")
            nc.sync.dma_start(wvt[:], wvt_d.rearrange("p (c v) -> p c v", c=DC))
            wch = constp.tile([128, VT], F32R, tag="wch")
            nc.sync.dma_start(wch[:], wch_d[:])
            ones = constp.tile([1, 128], F32R, tag="ones")
            nc.sync.dma_start(ones[:], ones_d[:])

            red = finp.tile([128, DC * NBLK * replicate], F32, tag="red")

            def emit_body(r):
                et_tiles = []
                for b in range(NBLK):
                    et_b = etp.tile([128, DC, BLK], F32R, tag="et")
                    nc.sync.dma_start(
                        et_b[:], etr_d[b].rearrange("p (c j) -> p c j", c=DC))
                    et_tiles.append(et_b)

                for b in range(NBLK):
                    et_b = et_tiles[b]
                    sc_ps = scp.tile([1, BLK], F32, tag="sc")

                    for vt in range(VT):
                        h = hp.tile([128, BLK], F32, tag="h")
                        for c in range(DC):
                            nc.tensor.matmul(
                                h[:],
                                wvt[:, c, vt * 128:(vt + 1) * 128],
                                et_b[:, c, :],
                                start=(c == 0),
                                stop=(c == DC - 1),
                            )
                        th = tanhp.tile([128, BLK], F32R, tag="tanh")
                        nc.scalar.activation(
                            th[:], h[:], mybir.ActivationFunctionType.Tanh)
                        nc.tensor.matmul(
                            sc_ps[:],
                            wch[:, vt:vt + 1],
                            th[:],
                            start=(vt == 0),
                            stop=(vt == VT - 1),
                        )

                    sexp_b = sexpp.tile([1, BLK], F32, tag="sexp")
                    nc.scalar.activation(
                        sexp_b[:], sc_ps[:], mybir.ActivationFunctionType.Exp)
                    nc.sync.dma_start(pw_d[b:b + 1, :], sexp_b[:])
                    sexpr_b = sexpp.tile([1, BLK], F32R, tag="sexpr")
                    nc.scalar.activation(
                        sexpr_b[:], sc_ps[:], mybir.ActivationFunctionType.Exp)

                    pbc = pbcp.tile([128, BLK], F32, tag="pbc")
                    nc.tensor.matmul(
                        pbc[:], ones[:], sexpr_b[:], start=True, stop=True)

                    for c in range(DC):
                        prod = scrp.tile([128, BLK], F32, tag="prod")
                        nc.vector.tensor_mul(
                            prod[:], et_b[:, c, :].bitcast(F32), pbc[:])
                        junk = scrp.tile([128, BLK], F32, tag="junk")
                        nc.scalar.activation(
                            junk[:], prod[:],
                            mybir.ActivationFunctionType.Copy,
                            accum_out=red[:, (r * NBLK + b) * DC + c:
                                          (r * NBLK + b) * DC + c + 1],
                        )

            if loop_iters > 1:
                assert replicate == 1
                with tc.For_i(0, loop_iters, 1):
                    emit_body(0)
            else:
                for r in range(replicate):
                    emit_body(r)

            pool_sb = finp.tile([128, DC], F32, tag="pool")
            nc.vector.tensor_reduce(
                pool_sb[:],
                red[:, 0:DC * NBLK * replicate].rearrange(
                    "p (b c) -> p c b", c=DC),
                axis=mybir.AxisListType.X,
                op=mybir.AluOpType.add,
            )
            nc.sync.dma_start(pool_d[:], pool_sb[:])

    nc.compile()
    return nc


def prepare_inputs(embeddings: np.ndarray, W_V: np.ndarray, w: np.ndarray):
    """Host-side sharding + layout prep. Returns per-core input maps."""
    E = np.ascontiguousarray(embeddings, dtype=np.float32)
    wvt = np.ascontiguousarray(
        W_V.T.astype(np.float32).reshape(DC, 128, V).transpose(1, 0, 2)
    ).reshape(128, DC * V)
    wvt = _tf32_round(wvt)
    # wch[p, c] = w[c*128 + p]
    wch = _tf32_round(np.ascontiguousarray(
        w.astype(np.float32).reshape(VT, 128).T))
    ones = np.ones((1, 128), dtype=np.float32)

    in_maps = []
    for c in range(NCORES):
        Ec = E[c * NC_ROWS:(c + 1) * NC_ROWS]  # [4096, 1024]
        # etr[b, p, cc, j] = Ec[b*512 + j, cc*128 + p]
        etr = np.ascontiguousarray(
            Ec.reshape(NBLK, BLK, DC, 128).transpose(0, 3, 2, 1)
        ).reshape(NBLK, 128, DC * BLK)
        etr = _tf32_round(etr)
        in_maps.append({"etr": etr, "wvt": wvt, "wch": wch, "ones": ones})
    return in_maps


def postprocess(results):
    """Combine per-core outputs into (attn_embedding, normalized_weights)."""
    p_all = np.concatenate(
        [results[c]["pw"].reshape(NC_ROWS) for c in range(NCORES)])
    pool_total = np.zeros(D, dtype=np.float32)
    for c in range(NCORES):
        pool_total += results[c]["pool"].T.reshape(D)
    S = p_all.sum(dtype=np.float32)
    weights = (p_all / S).astype(np.float32)
    attn = (pool_total / S).astype(np.float32)
    return attn, weights


_NC_CACHE = {}


def kernel(embeddings: np.ndarray, W_V: np.ndarray, w: np.ndarray):
    if "nc" not in _NC_CACHE:
        _NC_CACHE["nc"] = build_program()
    nc = _NC_CACHE["nc"]
    in_maps = prepare_inputs(embeddings, W_V, w)
    res = run_bass_kernel_spmd(nc, in_maps, core_ids=list(range(NCORES)))
    return postprocess(res.results)


# revision 25
# speedup vs baseline: 1.1686x; 1.1686x over previous
"""Attention-pooling kernel for 8 Trainium2 NeuronCores (SPMD, data-parallel over N).

Reference computation (fp32):
    h      = tanh(embeddings @ W_V.T)        [N, 512]
    scores = h @ w                           [N]
    p      = softmax(scores)                 [N]
    attn   = p @ embeddings                  [1024]
    returns (attn, p)

Strategy:
  - Shard N=32768 across 8 cores (4096 rows each).
  - Host pre-transposes each shard to E^T (d on partitions) so the score
    matmul needs no on-chip transpose; host pre-rounds PE operands to TF32
    (float32r) so the PE runs at full rate (1 cycle/row vs 4 for fp32).
  - Per core, per 512-instance block:
      h^T[v,i] = sum_d W_V^T[d,v] E^T[d,i]   PE, f32r, PSUM-accumulated
      tanh                                    ACT (psum -> sbuf, f32r out)
      scores[1,i] = sum_v w[v] h^T[v,i]       PE, f32r
      p = exp(scores)                         ACT
      p broadcast across 128 partitions       PE outer product with ones
      prod[d,i] = E^T[d,i] * p[i]             DVE
      partial[d] += sum_i prod[d,i]           ACT copy with accum_out
  - No collectives: each core returns its unnormalized exp(scores) slice and
    partial pooled sums; the tiny final reduction (8 x 1025 floats) happens
    on host during the gather step.
"""

import numpy as np

import concourse.bacc as bacc
import concourse.mybir as mybir
import concourse.tile as tile
from concourse.bass_utils import run_bass_kernel_spmd

F32 = mybir.dt.float32
F32R = mybir.dt.float32r

N = 32768
D = 1024
V = 512
NCORES = 8
NC_ROWS = N // NCORES          # 4096 instances per core
NBLK = 8                       # i-blocks per core
BLK = NC_ROWS // NBLK          # 512 instances per block
VT = V // 128                  # 4 v-tiles
DC = D // 128                  # 8 d-chunks


def _tf32_round(x: np.ndarray) -> np.ndarray:
    """Round fp32 to TF32 (float32r) bit patterns, round-to-nearest-even."""
    u = np.ascontiguousarray(x, dtype=np.float32).view(np.uint32).copy()
    lsb = (u >> np.uint32(13)) & np.uint32(1)
    u += np.uint32(0x0FFF) + lsb
    u &= np.uint32(0xFFFFE000)
    return u.view(np.float32)


def build_program(replicate: int = 1, loop_iters: int = 1, parts: str = "full",
                  etbufs: int = 6, hbufs: int = 3, ndve: int = 3,
                  scbufs: int = 2, pbcbufs: int = 2):
    """Build the SPMD single-core program (same program on all 8 cores).

    replicate > 1 unrolls the whole compute body; loop_iters > 1 wraps the
    body in a hardware For_i loop (same inputs, same outputs each iteration).
    Both are identity transforms on the output, used only for timing.
    """
    nc = bacc.Bacc("TRN2", target_bir_lowering=False, debug=False,
                   num_devices=NCORES)

    etr_d = nc.dram_tensor("etr", [NBLK, 128, DC * BLK], F32R,
                           kind="ExternalInput").ap()
    wvt_d = nc.dram_tensor("wvt", [128, DC * V], F32R,
                           kind="ExternalInput").ap()
    wch_d = nc.dram_tensor("wch", [128, VT], F32R, kind="ExternalInput").ap()
    ones_d = nc.dram_tensor("ones", [1, 128], F32R, kind="ExternalInput").ap()
    pool_d = nc.dram_tensor("pool", [128, DC], F32,
                            kind="ExternalOutput").ap()
    pw_d = nc.dram_tensor("pw", [NBLK, BLK], F32, kind="ExternalOutput").ap()

    with tile.TileContext(nc) as tc:
        with (
            tc.tile_pool(name="const", bufs=1) as constp,
            tc.tile_pool(name="et", bufs=etbufs) as etp,
            tc.tile_pool(name="tanh", bufs=9) as tanhp,
            tc.tile_pool(name="scr", bufs=3) as scrp,
            tc.tile_pool(name="pbsb", bufs=3) as pbsp,
            tc.tile_pool(name="sexp", bufs=4) as sexpp,
            tc.tile_pool(name="fin", bufs=1) as finp,
            tc.tile_pool(name="hps", bufs=hbufs, space="PSUM") as hp,
            tc.tile_pool(name="scps", bufs=scbufs, space="PSUM") as scp,
            tc.tile_pool(name="pbcps", bufs=pbcbufs, space="PSUM") as pbcp,
            tc.tile_pool(name="jk", bufs=1, space="PSUM") as jkp,
        ):
            wvt = constp.tile([128, DC, V], F32R, tag="wvt")
            nc.gpsimd.dma_start(wvt[:], wvt_d.rearrange("p (c v) -> p c v", c=DC))
            wch = constp.tile([128, VT], F32R, tag="wch")
            nc.gpsimd.dma_start(wch[:], wch_d[:])
            ones = constp.tile([1, 128], F32R, tag="ones")
            nc.gpsimd.dma_start(ones[:], ones_d[:])

            red = finp.tile([128, DC * NBLK * replicate], F32, tag="red")

            NDVE = ndve  # d-chunks multiplied on DVE; the rest go to GpSimd

            def emit_body(r):
                et_tiles = []
                for b in range(NBLK):
                    et_b = etp.tile([128, DC, BLK], F32R, tag="et")
                    src_b = etr_d[b].rearrange("p (c j) -> p c j", c=DC)
                    if b == 0 and r == 0:
                        nc.sync.dma_start(et_b[:, 0:1, :], src_b[:, 0:1, :])
                        nc.sync.dma_start(et_b[:, 1:2, :], src_b[:, 1:2, :])
                        nc.sync.dma_start(et_b[:, 2:4, :], src_b[:, 2:4, :])
                        nc.sync.dma_start(et_b[:, 4:DC, :], src_b[:, 4:DC, :])
                    elif b == 1 and r == 0:
                        nc.sync.dma_start(et_b[:, 0:4, :], src_b[:, 0:4, :])
                        nc.sync.dma_start(et_b[:, 4:DC, :], src_b[:, 4:DC, :])
                    else:
                        nc.sync.dma_start(et_b[:], src_b)
                    et_tiles.append(et_b)

                sc_tiles = {}
                th_tiles = {}
                sexp_tiles = {}
                pbc_tiles = {}

                def emit_h(b, vts):
                    # h matmuls + tanh for block b, v-tiles in vts
                    et_b = et_tiles[b]
                    if parts == "dmaOnly":
                        if 0 in vts:
                            junk4 = scrp.tile([128, 4], F32, tag="junk4")
                            nc.vector.tensor_copy(
                                junk4[:], et_b[:, 0, 0:4].bitcast(F32))
                        return
                    if 0 in vts:
                        th_tiles[b] = []
                    for vt in vts:
                        h = hp.tile([128, BLK], F32, tag="h")
                        for c in range(1 if parts == "hOnly1c" else DC):
                            nc.tensor.matmul(
                                h[:],
                                wvt[:, c, vt * 128:(vt + 1) * 128],
                                et_b[:, c, :],
                                start=(c == 0),
                                stop=(c == DC - 1) or parts == "hOnly1c",
                            )
                        if parts in ("hOnly", "hOnly1c"):
                            continue
                        th = tanhp.tile([128, BLK], F32R, tag="tanh")
                        nc.scalar.activation(
                            th[:], h[:], mybir.ActivationFunctionType.Tanh)
                        th_tiles[b].append(th)

                def emit_sc(b):
                    # score matmuls + exp for block b
                    if parts in ("hOnly", "hOnly1c", "hTanhOnly", "dmaOnly"):
                        return
                    sc_ps = scp.tile([1, BLK], F32, tag="sc", name="sc_ps")
                    sc_tiles[b] = sc_ps
                    for vt in range(VT):
                        nc.tensor.matmul(
                            sc_ps[:],
                            wch[:, vt:vt + 1],
                            th_tiles[b][vt][:],
                            start=(vt == 0),
                            stop=(vt == VT - 1),
                        )
                    sexpr_b = sexpp.tile([1, BLK], F32R, tag="sexpr")
                    nc.scalar.activation(
                        sexpr_b[:], sc_ps[:], mybir.ActivationFunctionType.Exp)
                    nc.sync.dma_start(pw_d[b:b + 1, :], sexpr_b[:].bitcast(F32))
                    sexp_tiles[b] = sexpr_b

                def emit_bcast(b):
                    # broadcast p across partitions (PE) + copy to SBUF (ACT)
                    if parts in ("hOnly", "hOnly1c", "hTanhOnly", "dmaOnly"):
                        return
                    pbc = pbcp.tile([128, BLK], F32, tag="pbc")
                    nc.tensor.matmul(
                        pbc[:], ones[:], sexp_tiles[b][:], start=True,
                        stop=True)
                    pbc_sb = pbsp.tile([128, BLK], F32, tag="pbc_sb")
                    nc.scalar.copy(pbc_sb[:], pbc[:])
                    pbc_tiles[b] = pbc_sb

                def emit_pool(b):
                    # weighted pooling for block b: mults on DVE + GpSimd,
                    # 3D reduce(s) on DVE
                    if parts in ("hOnly", "hOnly1c", "hTanhOnly", "dmaOnly",
                                 "noPool"):
                        return
                    et_b = et_tiles[b]
                    pbc_sb = pbc_tiles[b]
                    n_dve = 4
                    nred_dve = 6
                    prod = scrp.tile([128, DC, BLK], F32, tag="prod")
                    for c in range(DC):
                        eng = nc.vector if c < n_dve else nc.gpsimd
                        eng.tensor_mul(
                            prod[:, c, :], et_b[:, c, :].bitcast(F32),
                            pbc_sb[:])
                    base = (r * NBLK + b) * DC
                    red_v = red[:, base:base + DC].rearrange(
                        "p (c o) -> p c o", o=1)
                    nc.vector.tensor_reduce(
                        red_v[:, 0:n_dve], prod[:, 0:n_dve],
                        axis=mybir.AxisListType.X, op=mybir.AluOpType.add)
                    if nred_dve > n_dve:
                        nc.vector.tensor_reduce(
                            red_v[:, n_dve:nred_dve], prod[:, n_dve:nred_dve],
                            axis=mybir.AxisListType.X, op=mybir.AluOpType.add)
                    for c in range(nred_dve, DC):
                        junk = jkp.tile([128, BLK], F32, tag="junk")
                        nc.scalar.activation(
                            junk[:], prod[:, c, :],
                            mybir.ActivationFunctionType.Copy,
                            accum_out=red[:, base + c:base + c + 1])

                for b in range(NBLK):
                    emit_h(b, [0])
                    if b >= 1:
                        emit_sc(b - 1)
                    emit_h(b, [1, 2, 3])
                    if b >= 1:
                        emit_bcast(b - 1)
                        emit_pool(b - 1)
                emit_sc(NBLK - 1)
                emit_bcast(NBLK - 1)
                emit_pool(NBLK - 1)

            if loop_iters > 1:
                assert replicate == 1
                with tc.For_i(0, loop_iters, 1):
                    emit_body(0)
            else:
                for r in range(replicate):
                    emit_body(r)

            if parts != "full":
                nc.vector.memset(red[:], 0.0)
            pool_sb = finp.tile([128, DC], F32, tag="pool")
            nc.vector.tensor_reduce(
                pool_sb[:],
                red[:, 0:DC * NBLK * replicate].rearrange(
                    "p (b c) -> p c b", c=DC),
                axis=mybir.AxisListType.X,
                op=mybir.AluOpType.add,
            )
            nc.sync.dma_start(pool_d[:], pool_sb[:])

    nc.compile()
    return nc


def prepare_inputs(embeddings: np.ndarray, W_V: np.ndarray, w: np.ndarray):
    """Host-side sharding + layout prep. Returns per-core input maps."""
    E = np.ascontiguousarray(embeddings, dtype=np.float32)
    wvt = np.ascontiguousarray(
        W_V.T.astype(np.float32).reshape(DC, 128, V).transpose(1, 0, 2)
    ).reshape(128, DC * V)
    wvt = _tf32_round(wvt)
    # wch[p, c] = w[c*128 + p]
    wch = _tf32_round(np.ascontiguousarray(
        w.astype(np.float32).reshape(VT, 128).T))
    ones = np.ones((1, 128), dtype=np.float32)

    in_maps = []
    for c in range(NCORES):
        Ec = E[c * NC_ROWS:(c + 1) * NC_ROWS]  # [4096, 1024]
        # etr[b, p, cc, j] = Ec[b*512 + j, cc*128 + p]
        etr = np.ascontiguousarray(
            Ec.reshape(NBLK, BLK, DC, 128).transpose(0, 3, 2, 1)
        ).reshape(NBLK, 128, DC * BLK)
        etr = _tf32_round(etr)
        in_maps.append({"etr": etr, "wvt": wvt, "wch": wch, "ones": ones})
    return in_maps


def postprocess(results):
    """Combine per-core outputs into (attn_embedding, normalized_weights)."""
    p_all = np.concatenate(
        [results[c]["pw"].reshape(NC_ROWS) for c in range(NCORES)])
    pool_total = np.zeros(D, dtype=np.float32)
    for c in range(NCORES):
        pool_total += results[c]["pool"].T.reshape(D)
    S = p_all.sum(dtype=np.float32)
    weights = (p_all / S).astype(np.float32)
    attn = (pool_total / S).astype(np.float32)
    return attn, weights


_NC_CACHE = {}


def kernel(embeddings: np.ndarray, W_V: np.ndarray, w: np.ndarray):
    if "nc" not in _NC_CACHE:
        _NC_CACHE["nc"] = build_program()
    nc = _NC_CACHE["nc"]
    in_maps = prepare_inputs(embeddings, W_V, w)
    res = run_bass_kernel_spmd(nc, in_maps, core_ids=list(range(NCORES)))
    return postprocess(res.results)
